# revision 12
# baseline (speedup 1.0000x reference)
"""Trainium2 Bass kernel for ComiRec dynamic-routing (CapsNet-style) layer.

Problem: B=1024, S=200, E=128, C=128, n_caps=4, 3 routing rounds.

Sharding (8 cores): core i handles capsule n = i//2 and batch half h = i%2
(512 batch rows, processed as 4 chunks of 128 = the partition width).
Capsules/batches are independent -> no cross-core communication.

Per-core dataflow (v2):
  - W ([S,E,C] bf16, 13.1MB) is DMA'd ONCE and kept resident in SBUF
    ([e, s*c] layout, 51.2KB/partition).  bmt (masked behaviors, [e,s,b])
    streams per chunk in 8-s blocks.
  - stage A (PE): per s, u_s[b,c] = bmt_s^T @ W_s into PSUM ([b, 8C] groups),
    plus an interleaved accumulating matmul z0[b,c] = sum_s u_s.  The Act
    engine evacuates PSUM groups to SBUF as bf16 u[b,s,c].
  - round 0: caps0 = squash(z0 * invn)   (small ops)
  - rounds 1,2:
      delta[b,s] = sum_c u[b,s,c]*caps[b,c]  -- per-s fused multiply+reduce
        (tensor_tensor_reduce on DVE for s < ND, scalar_tensor_tensor on
        GpSimd for s >= ND), accumulated straight into delta columns.
      lm += delta  (mask folded in: lm starts as mneg = 0/-3e38)
      coup = exp(lm - max) via Act (fused accum -> Z), invz = 1/Z
      caps_pre = sum_s coup[b,s]*u[b,s,c]: DVE tensor_scalar writes
        coup-scaled 4-s groups into a small ring; the PE accumulates the
        ring tiles with a constant identity lhsT into one PSUM bank
        (out[b, j*C+c] += prod[b,4g+j,c]); 3 adds fold the 4 blocks.
      squash folded into per-partition scalars (sqrt on Act, rest DVE).
  - Emission is software-pipelined: stage A of chunk k+1 is emitted before
    the rounds of chunk k, so the PE/Act work of the next chunk overlaps the
    DVE/GpSimd round chain of the current one.

kernel() takes FULL inputs and returns the FULL [1024, 4, 128] fp32 output.
"""

import numpy as np
import ml_dtypes

BF16 = ml_dtypes.bfloat16

B, S, E, C, NCAPS = 1024, 200, 128, 128, 4
NCORES = 8
BH = B // 2          # batch rows per core
P = 128              # partition width / chunk size
NCHUNK = BH // P     # 4 chunks per core
SBLK = 8             # s per DMA block / PSUM group
NGRP = S // SBLK     # 25 groups
NEG = -3.0e38

ND = 200             # delta: s < ND on DVE, rest on GpSimd (tunable)
CAPS_G = 4           # s per caps-prod ring tile / identity-matmul
NROUNDS = 2          # debug: 0 = caps0 only, 1 = one routing round, 2 = full
USE_TTR = False       # use tensor_tensor_reduce (False: fallback ops)

_COMPILED = {}


def _emit(ctx, tc, nc):
    import concourse.bass as bass
    from concourse import mybir

    bf = mybir.dt.bfloat16
    f32 = mybir.dt.float32
    AF = mybir.ActivationFunctionType
    OP = mybir.AluOpType

    bmt = nc.dram_tensor("bmt", [NCHUNK, E, S, P], bf, kind="ExternalInput").ap()
    w = nc.dram_tensor("w", [E, S, C], bf, kind="ExternalInput").ap()
    ident = nc.dram_tensor("ident", [P, P], bf, kind="ExternalInput").ap()
    mneg = nc.dram_tensor("mneg", [NCHUNK, P, S], f32, kind="ExternalInput").ap()
    invn = nc.dram_tensor("invn", [NCHUNK, P, 1], f32, kind="ExternalInput").ap()
    out = nc.dram_tensor("caps_out", [NCHUNK, P, C], f32, kind="ExternalOutput").ap()

    # ---- persistent tiles ----
    wpool = ctx.enter_context(tc.tile_pool(name="wres", bufs=1))
    w_sb = wpool.tile([E, S, C], bf, tag="w")
    ident_sb = wpool.tile([P, P], bf, tag="ident")
    nc.sync.dma_start(out=ident_sb, in_=ident)
    # W resident; DMA in s-blocks so chunk-0 stage A can chase it.
    for g in range(NGRP):
        sl = slice(g * SBLK, (g + 1) * SBLK)
        nc.sync.dma_start(out=w_sb[:, sl, :], in_=w[:, sl, :])

    bmtpool = ctx.enter_context(tc.tile_pool(name="bmt", bufs=3))
    upool = ctx.enter_context(tc.tile_pool(name="u", bufs=2))
    prodpool = ctx.enter_context(tc.tile_pool(name="prod", bufs=4))
    pscrpool = ctx.enter_context(tc.tile_pool(name="pscr", bufs=2))
    smalls = ctx.enter_context(tc.tile_pool(name="smalls", bufs=2))
    pupool = ctx.enter_context(tc.tile_pool(name="pu", bufs=2, space="PSUM"))
    pzpool = ctx.enter_context(tc.tile_pool(name="pz", bufs=2, space="PSUM"))
    pcpool = ctx.enter_context(tc.tile_pool(name="pc", bufs=2, space="PSUM"))

    state = [None] * NCHUNK  # per-chunk dict of tiles

    def emit_stage_a(k):
        st = {}
        mneg_sb = smalls.tile([P, S], f32, tag="mneg")
        nc.sync.dma_start(out=mneg_sb, in_=mneg[k])
        invn_sb = smalls.tile([P, 1], f32, tag="invn")
        nc.sync.dma_start(out=invn_sb, in_=invn[k])
        st["mneg"] = mneg_sb
        st["invn"] = invn_sb

        u = upool.tile([P, S, C], bf, tag="u")
        st["u"] = u
        pz = pzpool.tile([P, C], f32, tag="pz")
        st["pz"] = pz

        for g in range(NGRP):
            sl = slice(g * SBLK, (g + 1) * SBLK)
            bt = bmtpool.tile([E, SBLK, P], bf, tag="bt")
            nc.sync.dma_start(out=bt, in_=bmt[k][:, sl, :])
            pu = pupool.tile([P, SBLK * C], f32, tag="pu")
            for j in range(SBLK):
                s = g * SBLK + j
                nc.tensor.matmul(
                    pu[:, j * C:(j + 1) * C],
                    lhsT=bt[:, j, :], rhs=w_sb[:, s, :],
                    start=True, stop=True,
                )
                nc.tensor.matmul(
                    pz, lhsT=bt[:, j, :], rhs=w_sb[:, s, :],
                    start=(s == 0), stop=(s == S - 1), skip_group_check=True,
                )
            pv = pu.rearrange("p (j c) -> p j c", c=C)
            nc.scalar.copy(u[:, sl, :], pv)
        return st

    def squash(st, zraw, invz, r):
        # caps = zraw * (invz^2 * sqrt(n2raw)) / (1 + invz^2 * n2raw)
        sq = smalls.tile([P, C], f32, tag="sq")
        n2r = smalls.tile([P, 1], f32, tag="n2r")
        if USE_TTR:
            nc.vector.tensor_tensor_reduce(
                out=sq, in0=zraw, in1=zraw, scale=1.0, scalar=0.0,
                op0=OP.mult, op1=OP.add, accum_out=n2r,
            )
        else:
            nc.scalar.activation(out=sq, in_=zraw, func=AF.Square, accum_out=n2r)
        a = smalls.tile([P, 1], f32, tag="a")
        nc.vector.tensor_mul(a, invz, invz)
        b_ = smalls.tile([P, 1], f32, tag="b")
        nc.vector.tensor_mul(b_, a, n2r)
        t = smalls.tile([P, 1], f32, tag="t")
        nc.scalar.sqrt(t, n2r)
        d = smalls.tile([P, 1], f32, tag="d")
        nc.vector.tensor_scalar_add(d, b_, 1.0)
        rcp = smalls.tile([P, 1], f32, tag="rcp")
        nc.vector.reciprocal(rcp, d)
        e = smalls.tile([P, 1], f32, tag="e")
        nc.vector.tensor_mul(e, a, t)
        f = smalls.tile([P, 1], f32, tag="f")
        nc.vector.tensor_mul(f, e, rcp)
        caps = smalls.tile([P, C], f32, tag=f"caps{r}")
        nc.vector.tensor_scalar_mul(caps, zraw, f)
        capsb = smalls.tile([P, C], bf, tag=f"capsb{r}")
        nc.vector.tensor_copy(capsb, caps)
        return caps, capsb

    def emit_rounds(k):
        st = state[k]
        u = st["u"]

        # ---- round 0: caps0 = squash(z0 * invn) ----
        zraw = smalls.tile([P, C], f32, tag="zraw0")
        nc.scalar.copy(zraw, st["pz"])
        caps, capsb = squash(st, zraw, st["invn"], 0)

        lm = smalls.tile([P, S], f32, tag="lm")
        delta = smalls.tile([P, S], f32, tag="delta")

        for r in range(1, NROUNDS + 1):
            # ---- delta[b,s] = sum_c u * caps ----
            dscr = smalls.tile([P, C], bf, tag="dscr")
            for s in range(ND):
                nc.vector.scalar_tensor_tensor(
                    out=dscr, in0=u[:, s, :], scalar=1.0, in1=capsb,
                    op0=OP.mult, op1=OP.mult, accum_out=delta[:, s:s + 1],
                )
            if ND < S:
                pscr = pscrpool.tile([P, C], bf, tag="pscr")
                for s in range(ND, S):
                    nc.gpsimd.scalar_tensor_tensor(
                        out=pscr, in0=u[:, s, :], scalar=1.0, in1=capsb,
                        op0=OP.mult, op1=OP.mult, accum_out=delta[:, s:s + 1],
                    )
            # ---- logits (mask folded) + softmax ----
            if r == 1:
                nc.vector.tensor_add(lm, st["mneg"], delta)
            else:
                nc.vector.tensor_add(lm, lm, delta)
            mx = smalls.tile([P, 1], f32, tag="mx")
            nc.vector.tensor_reduce(out=mx, in_=lm, axis=mybir.AxisListType.X,
                                    op=OP.max)
            negmx = smalls.tile([P, 1], f32, tag="negmx")
            nc.vector.tensor_scalar_mul(negmx, mx, -1.0)
            coup = smalls.tile([P, S], f32, tag="coup")
            zsum = smalls.tile([P, 1], f32, tag="zsum")
            nc.scalar.activation(out=coup, in_=lm, func=AF.Exp,
                                 bias=negmx, scale=1.0, accum_out=zsum)
            invz = smalls.tile([P, 1], f32, tag="invz")
            nc.vector.reciprocal(invz, zsum)

            # ---- caps_pre = sum_s coup*u : DVE scale + PE identity-MM ----
            pc = pcpool.tile([P, CAPS_G * C], f32, tag="pc")
            ngr = S // CAPS_G
            for g in range(ngr):
                prod = prodpool.tile([P, CAPS_G, C], bf, tag="prod")
                for j in range(CAPS_G):
                    s = g * CAPS_G + j
                    nc.vector.tensor_scalar_mul(
                        prod[:, j, :], u[:, s, :], coup[:, s:s + 1])
                nc.tensor.matmul(
                    pc, lhsT=ident_sb,
                    rhs=prod.rearrange("p j c -> p (j c)"),
                    start=(g == 0), stop=(g == ngr - 1), skip_group_check=True,
                )
            # fold 4 col-blocks (≤1 PSUM operand per tensor_tensor) + squash
            pcv = pc.rearrange("p (j c) -> p j c", c=C)
            zraw = smalls.tile([P, C], f32, tag=f"zraw{r}")
            nc.vector.tensor_copy(zraw, pcv[:, 0, :])
            for j in (1, 2, 3):
                nc.vector.tensor_add(zraw, zraw, pcv[:, j, :])
            caps, capsb = squash(st, zraw, invz, r)

        outsb = smalls.tile([P, C], f32, tag="outsb")
        nc.vector.tensor_copy(outsb, caps)
        nc.sync.dma_start(out=out[k], in_=outsb)

    # software-pipelined emission: stage A of k runs ahead of rounds of k-1
    for k in range(NCHUNK):
        state[k] = emit_stage_a(k)
        if k > 0:
            emit_rounds(k - 1)
    emit_rounds(NCHUNK - 1)


def _build():
    if "nc" in _COMPILED:
        return _COMPILED["nc"]
    from contextlib import ExitStack
    import concourse.bacc as bacc
    import concourse.tile as tile

    nc = bacc.Bacc(
        "TRN2", target_bir_lowering=False, debug=False, enable_asserts=False
    )
    with tile.TileContext(nc, trace_sim=False) as tc, ExitStack() as ctx:
        _emit(ctx, tc, nc)
    nc.compile()
    _COMPILED["nc"] = nc
    return nc


def make_in_maps(behaviors, valid_mask, W):
    behaviors = np.asarray(behaviors, dtype=np.float32)
    mask = np.asarray(valid_mask).astype(bool)
    W = np.asarray(W, dtype=np.float32)

    bm = behaviors * mask[:, :, None].astype(np.float32)
    # [B,S,E] -> [E,S,B] (e-major so each DMA row is contiguous)
    bmt_full = np.ascontiguousarray(bm.transpose(2, 1, 0)).astype(BF16)
    w_esc = np.ascontiguousarray(W.transpose(0, 2, 1, 3)).astype(BF16)  # [N,E,S,C]
    mneg_full = np.where(mask, 0.0, NEG).astype(np.float32)             # [B,S]
    nval = mask.sum(axis=1).astype(np.float32)
    invn_full = (1.0 / np.maximum(nval, 1.0)).astype(np.float32)        # [B]
    ident = np.eye(P, dtype=np.float32).astype(BF16)

    in_maps = []
    for core in range(NCORES):
        n, h = core // 2, core % 2
        bsl = slice(h * BH, (h + 1) * BH)
        bmt_h = bmt_full[:, :, bsl]                                     # [E,S,BH]
        bmt_c = np.ascontiguousarray(
            bmt_h.reshape(E, S, NCHUNK, P).transpose(2, 0, 1, 3))       # [K,E,S,P]
        in_maps.append({
            "bmt": bmt_c,
            "w": w_esc[n],
            "ident": ident,
            "mneg": np.ascontiguousarray(mneg_full[bsl].reshape(NCHUNK, P, S)),
            "invn": np.ascontiguousarray(invn_full[bsl].reshape(NCHUNK, P, 1)),
        })
    return in_maps


def gather_output(results):
    out = np.empty((B, NCAPS, C), dtype=np.float32)
    for core in range(NCORES):
        n, h = core // 2, core % 2
        caps = results[core]["caps_out"].reshape(BH, C)
        out[h * BH:(h + 1) * BH, n, :] = caps
    return out


def kernel(behaviors, valid_mask, W):
    from concourse import bass_utils

    nc = _build()
    in_maps = make_in_maps(behaviors, valid_mask, W)
    res = bass_utils.run_bass_kernel_spmd(nc, in_maps, core_ids=list(range(NCORES)))
    return gather_output(res.results)
